# revision 48
# baseline (speedup 1.0000x reference)
"""Trainium2 Bass kernel for soft decision-tree histogram binning.

Computes out[b, j] = prod_f softmax(x[b,f]*W + b_f, T=0.1)[digit_f(j)]
for x (4096, 7), cutpoints (7, 3) -> out (4096, 4**7=16384) float32.

Strategy (data-parallel over batch, 8 cores x 512 rows):
  - per-feature bias b_f from a 3-element min/mid/max sort of cutpoints,
    computed redundantly on all 128 partitions (no cross-partition traffic)
  - stabilized unnormalized e = exp((h - max_d h)/T) on the tiny (128, 28)
    tile; all 7 softmax denominators folded into one per-row scale
    C = 1/prod_f Z_f, applied once at the 256-wide cascade level so the
    last-level scale table sc16 = e1 (x) e0 has no reciprocal dependency
  - output built as a Kronecker cascade (4 -> 16 -> 64 -> 256 via
    double-broadcast tensor_tensor, -> 1024 via tensor_scalar); cascade
    stored bf16 so the hot final scale-ops run in DVE 4x mode
  - final scale-ops split ~3:1 DVE:ScalarE; sc16/s4 side-chain on GpSimd
  - output stored bf16 (upcast on host): 16 MiB/core HBM write drain.
    Rounding ~2^-9/elem, normed rel err ~1.6e-3 vs the 2e-2 harness gate
  - each DMA block written to a CONTIGUOUS flat DRAM range (host unshards
    via _BLOCK_PLAN): sequential HBM writes measure ~390-405 GB/s vs
    ~340 GB/s for 32 KiB-strided rows
  - ramp engineering: input DMA hoisted to the top of the NEFF entry block
    (dispatches at the Sync queue's ~6.8us floor, ahead of the const-AP
    memsets and entry barrier; no sem-clear exists in this mode so its
    completion increments are safe), 512 B input lines (SDMA line-rate
    threshold), tile 0 leads with 256/256/512-col pieces sourced from the
    256-wide cascade level, and h/e are single-buffered so tile t+1's
    front chain can't crowd tile t's first blocks off the Vector queue
  - front-chain temporaries are allocated once and overwritten per tile:
    fewer pool cap-acquires (lower per-op dispatch overhead, shorter
    end-of-kernel semaphore drain), and the WAR dependencies also
    serialize consecutive tiles' front chains
  - measured (core 0): best ~56.4 us (405+ GB/s runs); contended runs
    ~60-67 us when the paired NC saturates the shared 716 GB/s HBM stack
"""

import numpy as np

B = 4096
F = 7
D1 = 4  # D+1 bins per feature
OUT = D1**F  # 16384
NCORES = 8
ROWS = B // NCORES  # 512
P = 128
NTILES = ROWS // P  # 4
INV_T = 10.0

# Per-tile DMA block plan as (c0, c1) column ranges of the (P, OUT) tile,
# in emission order. The device writes blocks back-to-back into a flat
# DRAM tensor (each block contiguous: partition-major [P, c1-c0]); the
# host unshard scatters them back into (ROWS, OUT). Tile 0 leads with
# four 256-col pieces sourced straight from the 256-wide cascade level
# so the write stream starts before the 1024-wide level even exists.
_BLOCK_PLAN = []
for _t in range(NTILES):
    if _t == 0:
        _blocks = [(0, 256), (256, 512), (512, 1024)]
        _u = 1
        for _n in (2, 2, 2, 2, 2, 2, 2, 1):
            _blocks.append((_u * 1024, (_u + _n) * 1024))
            _u += _n
    elif _t < NTILES - 1:
        _blocks = [(0, 8192), (8192, 16384)]
    else:
        # last tile: smaller final block so the end-of-kernel completion
        # receipt starts sooner after the last byte
        _blocks = [(0, 8192), (8192, 12288), (12288, 16384)]
    _BLOCK_PLAN.append(_blocks)

# units (1024-col chunks) handled by ScalarE instead of DVE, per tile
_SCALAR_UNITS = [{6, 10, 14}, {2, 6, 10, 14}, {2, 6, 10, 14}, {2, 6, 10, 14}]

_cache = {}


def _build_bass():
    import concourse.bacc as bacc
    import concourse.tile as tile
    from concourse import mybir

    f32 = mybir.dt.float32
    bf16 = mybir.dt.bfloat16
    Alu = mybir.AluOpType
    Act = mybir.ActivationFunctionType
    AX = mybir.AxisListType.X

    from concourse.vector_clock import ScopedClock

    class LeanTileContext(tile.TileContext):
        """TileContext with a minimal kernel exit: keep the sync-engine
        drain that waits for all outstanding work (so the NEFF cannot
        complete with DMAs in flight), skip the two all-engine barriers
        and the semaphore recycle loop. Each kernel() call compiles and
        loads a fresh NEFF, so semaphores never need to be handed back."""

        def _drain_and_barrier(self, tick_clock, wait_clock):
            drain_inst = self.nc.sync.drain()
            wait_clock.add_sem_waits(
                drain_inst.ins, ScopedClock({None: tick_clock.global_clock})
            )
            popped = self.nc._tile_sem_poison_stack.pop()
            assert popped is self._sem_poison

    nc = bacc.Bacc("TRN2", target_bir_lowering=False, debug=False)

    # xw[p, :] = [x rows {p,128+p,256+p,384+p} (28) | W pattern (28) | cutpoints (21)]
    # padded to 128 f32 = 512 B per partition line (SDMA line-rate threshold)
    XWC = NTILES * F + F * D1 + F * 3  # 77
    XWP = 128
    xw_d = nc.dram_tensor("xw", [P, XWP], f32, kind="ExternalInput").ap()
    out_d = nc.dram_tensor("out", [ROWS * OUT], bf16, kind="ExternalOutput").ap()

    # Input DMA issued in the entry block, before the TileContext branch, so
    # it overlaps the ~7 us engine prologue (library/act-table loads) instead
    # of queueing behind it. Only Vector reads xw directly, so only Vector
    # gates on the completion semaphore; all other engines' dependencies
    # flow through tile-tracked tensors produced by Vector.
    # Input DMA issued in the entry block (hoisted below), gated by a raw
    # completion semaphore with the wait on Vector's pre-branch position.
    # Note: an in-context tile-tracked input DMA was also tried — it makes
    # the branch ungated (absorbing the ~1.4us body-fetch stall during the
    # input flight) but its framework-tracked completion chain is ~1.3us
    # slower than the raw then_inc path; the two designs measure equal at
    # first-output-DMA and this one has the shorter exit drain.
    xw_sb = nc.alloc_sbuf_tensor("xw_sb", [P, XWP], f32)
    in_sem = nc.alloc_semaphore("in_sem")
    nc.sync.dma_start(out=xw_sb.ap(), in_=xw_d).then_inc(in_sem, 16)
    nc.vector.wait_ge(in_sem, 16)

    with LeanTileContext(nc) as tc:
        with (
            tc.tile_pool(name="hser", bufs=1) as hp,
            tc.tile_pool(name="small", bufs=2) as sp,
            tc.tile_pool(name="mid", bufs=2) as mp,
            tc.tile_pool(name="blk", bufs=8) as blkp,
        ):
            xw = xw_sb.ap()
            x_all = xw[:, 0 : NTILES * F]
            w4 = xw[:, NTILES * F : NTILES * F + F * D1].rearrange(
                "p (f d) -> p f d", d=D1
            )
            cp3 = xw[:, NTILES * F + F * D1 : XWC].rearrange("p (f c) -> p f c", c=3)

            # b_f = [0, -min, max-sum, -sum] per feature (cumsum of -sorted cuts)
            vmax = sp.tile([P, F], f32, tag="vmax")
            brep = sp.tile([P, F * D1], f32, tag="brep")
            # front-chain temporaries allocated once and overwritten per
            # tile: fewer pool cap-acquires (less per-op dispatch overhead
            # and a shorter end-of-kernel semaphore drain), and the WAR
            # dependencies double as cross-tile serialization of the fronts
            m7 = sp.tile([P, F], f32, tag="m7")
            z7 = sp.tile([P, F], f32, tag="z7")
            zp = sp.tile([P, 1], f32, tag="zp")
            c1 = sp.tile([P, 1], f32, tag="c1")
            sc16 = sp.tile([P, 16], f32, tag="sc16")
            s4 = sp.tile([P, D1], f32, tag="s4")
            t2 = sp.tile([P, 16], bf16, tag="t2")
            t3 = sp.tile([P, 64], bf16, tag="t3")
            t4r = sp.tile([P, 256], bf16, tag="t4r")
            t4 = sp.tile([P, 256], bf16, tag="t4")
            b4 = brep.rearrange("p (f d) -> p f d", d=D1)
            nc.vector.memset(b4[:, :, 0], 0.0)
            nc.vector.tensor_reduce(out=b4[:, :, 1], in_=cp3, axis=AX, op=Alu.min, negate=True)
            nc.vector.tensor_reduce(out=b4[:, :, 3], in_=cp3, axis=AX, op=Alu.add, negate=True)
            nc.vector.tensor_reduce(out=vmax, in_=cp3, axis=AX, op=Alu.max)
            nc.vector.tensor_tensor(out=b4[:, :, 2], in0=vmax, in1=b4[:, :, 3], op=Alu.add)

            out_off = 0
            for t in range(NTILES):
                xt = x_all[:, t * F : (t + 1) * F]

                # h[p, f, d] = x[p,f]*W[d] + b[f,d]. Single-buffered (bufs=1)
                # on purpose: tile t+1's front chain can't start until tile
                # t's exp consumed h, which keeps the scheduler from slotting
                # the next tile's small ops ahead of this tile's first DMAs.
                h = hp.tile([P, F * D1], f32, tag="h")
                h4 = h.rearrange("p (f d) -> p f d", d=D1)
                xb = xt[:, :, None].broadcast_to((P, F, D1))
                nc.vector.tensor_tensor(out=h4, in0=xb, in1=w4, op=Alu.mult)
                nc.vector.tensor_tensor(out=h4, in0=h4, in1=b4, op=Alu.add)

                # stabilize: h -= max_d h
                nc.vector.tensor_reduce(out=m7, in_=h4, axis=AX, op=Alu.max)
                mb = m7[:, :, None].broadcast_to((P, F, D1))
                nc.vector.tensor_tensor(out=h4, in0=h4, in1=mb, op=Alu.subtract)

                # e = exp(h / T), entries in (0, 1]. Also single-buffered: its
                # last consumer is this tile's t5 build, so the next tile's
                # exp (and everything downstream of it) can't crowd this
                # tile's lead blocks off the Vector queue.
                e = hp.tile([P, F * D1], f32, tag="e")
                nc.scalar.activation(out=e, in_=h, func=Act.Exp, scale=INV_T)
                e4 = e.rearrange("p (f d) -> p f d", d=D1)

                # z7 = per-feature sums, zp = prod of sums (Vector: GpSimd
                # can't do free-axis reductions); sc16 = e1 (x) e0 WITHOUT
                # the 1/prod scale — that scale is folded into t4 below, so
                # sc16/s4 (GpSimd) have no dependency on the reciprocal.
                nc.vector.tensor_reduce(out=z7, in_=e4, axis=AX, op=Alu.add)
                nc.vector.tensor_reduce(out=zp, in_=z7, axis=AX, op=Alu.mult)
                nc.gpsimd.tensor_tensor(
                    out=sc16.rearrange("p (a b) -> p a b", b=D1),
                    in0=e[:, 4:8, None].broadcast_to((P, D1, D1)),
                    in1=e[:, None, 0:4].broadcast_to((P, D1, D1)),
                    op=Alu.mult,
                )
                if t == 0:
                    # s4[p, d2] = sc16[p, 0] * e2[p, d2] — scalars for the
                    # lead pieces (d0 = d1 = 0)
                    nc.gpsimd.tensor_scalar_mul(
                        out=s4, in0=e[:, 8 : 8 + D1], scalar1=sc16[:, 0:1]
                    )

                # ---- Kronecker cascade: features 6,5 -> ... -> 1, then 0.
                # t2/t3/t4 as one double-broadcast tensor_tensor each. Cascade
                # levels are stored bf16 so the hot final scale-ops stream
                # bf16-in/bf16-out and hit DVE 4x mode (4 elem/cyc/lane).
                nc.vector.tensor_tensor(
                    out=t2.rearrange("p (a b) -> p a b", b=D1),
                    in0=e[:, 20:24, None].broadcast_to((P, D1, D1)),
                    in1=e[:, None, 24:28].broadcast_to((P, D1, D1)),
                    op=Alu.mult,
                )
                nc.vector.tensor_tensor(
                    out=t3.rearrange("p (a b) -> p a b", b=16),
                    in0=e[:, 16:20, None].broadcast_to((P, D1, 16)),
                    in1=t2[:, None, :].broadcast_to((P, D1, 16)),
                    op=Alu.mult,
                )
                nc.vector.tensor_tensor(
                    out=t4r.rearrange("p (a b) -> p a b", b=64),
                    in0=e[:, 12:16, None].broadcast_to((P, D1, 64)),
                    in1=t3[:, None, :].broadcast_to((P, D1, 64)),
                    op=Alu.mult,
                )
                # fold the softmax normalizer C = 1/prod_f Z_f into t4
                nc.vector.reciprocal(out=c1, in_=zp)
                nc.vector.tensor_scalar_mul(out=t4, in0=t4r, scalar1=c1)

                blocks = _BLOCK_PLAN[t]
                lead = [b for b in blocks if b[1] <= 1024]
                for i, (c0, c1_) in enumerate(lead):
                    L = c1_ - c0
                    lb = blkp.tile([P, L], bf16, tag=f"lead{L}")
                    for j in range(L // 256):
                        d2 = c0 // 256 + j
                        q = lb[:, j * 256 : (j + 1) * 256]
                        if d2 == 3:
                            nc.scalar.mul(out=q, in_=t4, mul=s4[:, d2 : d2 + 1])
                        else:
                            nc.vector.tensor_scalar_mul(
                                out=q, in0=t4, scalar1=s4[:, d2 : d2 + 1]
                            )
                    nc.sync.dma_start(
                        out=out_d[out_off : out_off + P * L].rearrange(
                            "(p l) -> p l", l=L
                        ),
                        in_=lb,
                    )
                    out_off += P * L

                t5 = mp.tile([P, 1024], bf16, tag="t5")
                for d in range(D1):
                    nc.vector.tensor_scalar_mul(
                        out=t5[:, d * 256 : (d + 1) * 256],
                        in0=t4,
                        scalar1=e[:, 8 + d : 9 + d],
                    )
                # remaining blocks: 1024-col units of t5 * sc16-col scale-ops,
                # DMA'd as soon as each block lands. DVE in 4x mode does a
                # 1024-elem unit in ~330ns vs ScalarE's ~1150ns => ~3:1 split.
                for c0, c1_ in blocks[len(lead) :]:
                    L = c1_ - c0
                    blk = blkp.tile([P, L], bf16, tag="blk")
                    for s in range(L // 1024):
                        u = c0 // 1024 + s
                        d0, d1 = u // D1, u % D1
                        scol = sc16[:, d1 * D1 + d0 : d1 * D1 + d0 + 1]
                        q = blk[:, s * 1024 : (s + 1) * 1024]
                        if u in _SCALAR_UNITS[t]:
                            nc.scalar.mul(out=q, in_=t5, mul=scol)
                        else:
                            nc.vector.tensor_scalar_mul(out=q, in0=t5, scalar1=scol)
                    nc.sync.dma_start(
                        out=out_d[out_off : out_off + P * L].rearrange(
                            "(p l) -> p l", l=L
                        ),
                        in_=blk,
                    )
                    out_off += P * L
    # Hoist the input DMA to the top of the entry block, ahead of the
    # const-AP memsets and the entry all-engine barrier. No semaphore-clear
    # exists in this mode (target_bir_lowering=False), so the completion
    # increments on in_sem cannot be wiped.
    entry = nc.main_func.blocks[0]
    dma_idx = next(
        i
        for i, ins in enumerate(entry.instructions)
        if type(ins).__name__ == "InstDMACopy"
    )
    entry.instructions.insert(0, entry.instructions.pop(dma_idx))

    nc.compile()
    return nc


def gather_core(flat):
    """Scatter one core's flat block stream back to its (ROWS, OUT) shard."""
    out = np.empty((ROWS, OUT), dtype=np.float32)
    flat = np.asarray(flat)
    off = 0
    for t, blocks in enumerate(_BLOCK_PLAN):
        for c0, c1 in blocks:
            L = c1 - c0
            out[t * P : (t + 1) * P, c0:c1] = (
                flat[off : off + P * L].reshape(P, L).astype(np.float32)
            )
            off += P * L
    return out


def build_in_maps(x, cutpoints):
    XWC = NTILES * F + F * D1 + F * 3
    XWP = 128
    wpat = np.tile(np.arange(1.0, D1 + 1.0, dtype=np.float32), F)
    cflat = cutpoints.ravel().astype(np.float32)
    # x sharded: core k, partition p gets rows k*512 + {p, 128+p, 256+p, 384+p}
    xs = (
        x.reshape(NCORES, NTILES, P, F)
        .transpose(0, 2, 1, 3)
        .reshape(NCORES, P, NTILES * F)
    )
    in_maps = []
    for k in range(NCORES):
        xw = np.zeros((P, XWP), dtype=np.float32)
        xw[:, 0 : NTILES * F] = xs[k]
        xw[:, NTILES * F : NTILES * F + F * D1] = wpat
        xw[:, NTILES * F + F * D1 : XWC] = cflat
        in_maps.append({"xw": xw})
    return in_maps


def kernel(x, cutpoints):
    from concourse import bass_utils

    if "nc" not in _cache:
        _cache["nc"] = _build_bass()
    nc = _cache["nc"]

    x = np.ascontiguousarray(np.asarray(x), dtype=np.float32)
    cutpoints = np.ascontiguousarray(np.asarray(cutpoints), dtype=np.float32)
    in_maps = build_in_maps(x, cutpoints)
    res = bass_utils.run_bass_kernel_spmd(nc, in_maps, list(range(NCORES))).results
    return np.concatenate([gather_core(res[k]["out"]) for k in range(NCORES)], axis=0)



# revision 49
# speedup vs baseline: 1.0007x; 1.0007x over previous
"""Trainium2 Bass kernel for soft decision-tree histogram binning.

Computes out[b, j] = prod_f softmax(x[b,f]*W + b_f, T=0.1)[digit_f(j)]
for x (4096, 7), cutpoints (7, 3) -> out (4096, 4**7=16384) float32.

Strategy (data-parallel over batch, 8 cores x 512 rows):
  - per-feature bias b_f from a 3-element min/mid/max sort of cutpoints,
    computed redundantly on all 128 partitions (no cross-partition traffic)
  - stabilized unnormalized e = exp((h - max_d h)/T) on the tiny (128, 28)
    tile; all 7 softmax denominators folded into one per-row scale
    C = 1/prod_f Z_f, applied once at the 256-wide cascade level so the
    last-level scale table sc16 = e1 (x) e0 has no reciprocal dependency
  - output built as a Kronecker cascade (4 -> 16 -> 64 -> 256 via
    double-broadcast tensor_tensor, -> 1024 via tensor_scalar); cascade
    stored bf16 so the hot final scale-ops run in DVE 4x mode
  - final scale-ops split ~3:1 DVE:ScalarE; sc16/s4 side-chain on GpSimd
  - output stored bf16 (upcast on host): 16 MiB/core HBM write drain.
    Rounding ~2^-9/elem, normed rel err ~1.6e-3 vs the 2e-2 harness gate
  - each DMA block written to a CONTIGUOUS flat DRAM range (host unshards
    via _BLOCK_PLAN): sequential HBM writes measure ~390-405 GB/s vs
    ~340 GB/s for 32 KiB-strided rows
  - ramp engineering: input DMA hoisted to the top of the NEFF entry block
    (dispatches at the Sync queue's ~6.8us floor, ahead of the const-AP
    memsets and entry barrier; no sem-clear exists in this mode so its
    completion increments are safe), 512 B input lines (SDMA line-rate
    threshold), tile 0 leads with 256/256/512-col pieces sourced from the
    256-wide cascade level, and h/e are single-buffered so tile t+1's
    front chain can't crowd tile t's first blocks off the Vector queue
  - front-chain temporaries are allocated once and overwritten per tile:
    fewer pool cap-acquires (lower per-op dispatch overhead, shorter
    end-of-kernel semaphore drain), and the WAR dependencies also
    serialize consecutive tiles' front chains
  - measured (core 0): best ~56.4 us (405+ GB/s runs); contended runs
    ~60-67 us when the paired NC saturates the shared 716 GB/s HBM stack
"""

import numpy as np

B = 4096
F = 7
D1 = 4  # D+1 bins per feature
OUT = D1**F  # 16384
NCORES = 8
ROWS = B // NCORES  # 512
P = 128
NTILES = ROWS // P  # 4
INV_T = 10.0

# Per-tile DMA block plan as (c0, c1) column ranges of the (P, OUT) tile,
# in emission order. The device writes blocks back-to-back into a flat
# DRAM tensor (each block contiguous: partition-major [P, c1-c0]); the
# host unshard scatters them back into (ROWS, OUT). Tile 0 leads with
# four 256-col pieces sourced straight from the 256-wide cascade level
# so the write stream starts before the 1024-wide level even exists.
_BLOCK_PLAN = []
for _t in range(NTILES):
    if _t == 0:
        _blocks = [(0, 256), (256, 512), (512, 1024)]
        _u = 1
        for _n in (2, 2, 2, 2, 2, 2, 2, 1):
            _blocks.append((_u * 1024, (_u + _n) * 1024))
            _u += _n
    elif _t < NTILES - 1:
        _blocks = [(0, 8192), (8192, 16384)]
    else:
        # last tile: smaller final block so the end-of-kernel completion
        # receipt starts sooner after the last byte
        _blocks = [(0, 8192), (8192, 12288), (12288, 16384)]
    _BLOCK_PLAN.append(_blocks)

# units (1024-col chunks) handled by ScalarE instead of DVE, per tile
_SCALAR_UNITS = [{6, 10, 14}, {2, 6, 10, 14}, {2, 6, 10, 14}, {2, 6, 10, 14}]

_cache = {}


def _build_bass():
    import concourse.bacc as bacc
    import concourse.tile as tile
    from concourse import mybir

    f32 = mybir.dt.float32
    bf16 = mybir.dt.bfloat16
    Alu = mybir.AluOpType
    Act = mybir.ActivationFunctionType
    AX = mybir.AxisListType.X

    from concourse.vector_clock import ScopedClock

    class LeanTileContext(tile.TileContext):
        """TileContext with a minimal kernel exit: keep the sync-engine
        drain that waits for all outstanding work (so the NEFF cannot
        complete with DMAs in flight), skip the two all-engine barriers
        and the semaphore recycle loop. Each kernel() call compiles and
        loads a fresh NEFF, so semaphores never need to be handed back."""

        def _drain_and_barrier(self, tick_clock, wait_clock):
            drain_inst = self.nc.sync.drain()
            wait_clock.add_sem_waits(
                drain_inst.ins, ScopedClock({None: tick_clock.global_clock})
            )
            popped = self.nc._tile_sem_poison_stack.pop()
            assert popped is self._sem_poison

    nc = bacc.Bacc("TRN2", target_bir_lowering=False, debug=False)

    # xw[p, :] = [x rows {p,128+p,256+p,384+p} (28) | W pattern (28) | cutpoints (21)]
    # padded to 128 f32 = 512 B per partition line (SDMA line-rate threshold)
    XWC = NTILES * F + F * D1 + F * 3  # 77
    XWP = 128
    xw_d = nc.dram_tensor("xw", [P, XWP], f32, kind="ExternalInput").ap()
    out_d = nc.dram_tensor("out", [ROWS * OUT], bf16, kind="ExternalOutput").ap()

    # Input DMA issued in the entry block, before the TileContext branch, so
    # it overlaps the ~7 us engine prologue (library/act-table loads) instead
    # of queueing behind it. Only Vector reads xw directly, so only Vector
    # gates on the completion semaphore; all other engines' dependencies
    # flow through tile-tracked tensors produced by Vector.
    # Input DMA issued in the entry block (hoisted below), gated by a raw
    # completion semaphore with the wait on Vector's pre-branch position.
    # Note: an in-context tile-tracked input DMA was also tried — it makes
    # the branch ungated (absorbing the ~1.4us body-fetch stall during the
    # input flight) but its framework-tracked completion chain is ~1.3us
    # slower than the raw then_inc path; the two designs measure equal at
    # first-output-DMA and this one has the shorter exit drain.
    xw_sb = nc.alloc_sbuf_tensor("xw_sb", [P, XWP], f32)
    in_sem = nc.alloc_semaphore("in_sem")
    nc.sync.dma_start(out=xw_sb.ap(), in_=xw_d).then_inc(in_sem, 16)
    nc.vector.wait_ge(in_sem, 16)

    with LeanTileContext(nc) as tc:
        with (
            tc.tile_pool(name="hser", bufs=1) as hp,
            tc.tile_pool(name="small", bufs=2) as sp,
            tc.tile_pool(name="mid", bufs=2) as mp,
            tc.tile_pool(name="blk", bufs=8) as blkp,
        ):
            xw = xw_sb.ap()
            x_all = xw[:, 0 : NTILES * F]
            w4 = xw[:, NTILES * F : NTILES * F + F * D1].rearrange(
                "p (f d) -> p f d", d=D1
            )
            cp3 = xw[:, NTILES * F + F * D1 : XWC].rearrange("p (f c) -> p f c", c=3)

            # b_f = [0, -min, max-sum, -sum] per feature (cumsum of -sorted cuts)
            vmax = sp.tile([P, F], f32, tag="vmax")
            brep = sp.tile([P, F * D1], f32, tag="brep")
            # front-chain temporaries allocated once and overwritten per
            # tile: fewer pool cap-acquires (less per-op dispatch overhead
            # and a shorter end-of-kernel semaphore drain), and the WAR
            # dependencies double as cross-tile serialization of the fronts
            m7 = sp.tile([P, F], f32, tag="m7")
            z7 = sp.tile([P, F], f32, tag="z7")
            zp = sp.tile([P, 1], f32, tag="zp")
            c1 = sp.tile([P, 1], f32, tag="c1")
            sc16 = sp.tile([P, 16], f32, tag="sc16")
            s4 = sp.tile([P, D1], f32, tag="s4")
            t2 = sp.tile([P, 16], bf16, tag="t2")
            t3 = sp.tile([P, 64], bf16, tag="t3")
            t4r = sp.tile([P, 256], bf16, tag="t4r")
            t4 = sp.tile([P, 256], bf16, tag="t4")
            b4 = brep.rearrange("p (f d) -> p f d", d=D1)
            nc.vector.memset(b4[:, :, 0], 0.0)
            nc.vector.tensor_reduce(out=b4[:, :, 1], in_=cp3, axis=AX, op=Alu.min, negate=True)
            nc.vector.tensor_reduce(out=b4[:, :, 3], in_=cp3, axis=AX, op=Alu.add, negate=True)
            nc.vector.tensor_reduce(out=vmax, in_=cp3, axis=AX, op=Alu.max)
            nc.vector.tensor_tensor(out=b4[:, :, 2], in0=vmax, in1=b4[:, :, 3], op=Alu.add)

            out_off = 0
            for t in range(NTILES):
                xt = x_all[:, t * F : (t + 1) * F]

                # h[p, f, d] = x[p,f]*W[d] + b[f,d]. Single-buffered (bufs=1)
                # on purpose: tile t+1's front chain can't start until tile
                # t's exp consumed h, which keeps the scheduler from slotting
                # the next tile's small ops ahead of this tile's first DMAs.
                h = hp.tile([P, F * D1], f32, tag="h")
                h4 = h.rearrange("p (f d) -> p f d", d=D1)
                xb = xt[:, :, None].broadcast_to((P, F, D1))
                nc.vector.tensor_tensor(out=h4, in0=xb, in1=w4, op=Alu.mult)
                nc.vector.tensor_tensor(out=h4, in0=h4, in1=b4, op=Alu.add)

                # stabilize: h -= max_d h
                nc.vector.tensor_reduce(out=m7, in_=h4, axis=AX, op=Alu.max)
                mb = m7[:, :, None].broadcast_to((P, F, D1))
                nc.vector.tensor_tensor(out=h4, in0=h4, in1=mb, op=Alu.subtract)

                # e = exp(h / T), entries in (0, 1]. Also single-buffered: its
                # last consumer is this tile's t5 build, so the next tile's
                # exp (and everything downstream of it) can't crowd this
                # tile's lead blocks off the Vector queue.
                e = hp.tile([P, F * D1], f32, tag="e")
                nc.scalar.activation(out=e, in_=h, func=Act.Exp, scale=INV_T)
                e4 = e.rearrange("p (f d) -> p f d", d=D1)

                # z7 = per-feature sums, zp = prod of sums (Vector: GpSimd
                # can't do free-axis reductions); sc16 = e1 (x) e0 WITHOUT
                # the 1/prod scale — that scale is folded into t4 below, so
                # sc16/s4 (GpSimd) have no dependency on the reciprocal.
                nc.vector.tensor_reduce(out=z7, in_=e4, axis=AX, op=Alu.add)
                nc.vector.tensor_reduce(out=zp, in_=z7, axis=AX, op=Alu.mult)
                nc.gpsimd.tensor_tensor(
                    out=sc16.rearrange("p (a b) -> p a b", b=D1),
                    in0=e[:, 4:8, None].broadcast_to((P, D1, D1)),
                    in1=e[:, None, 0:4].broadcast_to((P, D1, D1)),
                    op=Alu.mult,
                )
                if t == 0:
                    # s4[p, d2] = sc16[p, 0] * e2[p, d2] — scalars for the
                    # lead pieces (d0 = d1 = 0)
                    nc.gpsimd.tensor_scalar_mul(
                        out=s4, in0=e[:, 8 : 8 + D1], scalar1=sc16[:, 0:1]
                    )

                # ---- Kronecker cascade: features 6,5 -> ... -> 1, then 0.
                # t2/t3/t4 as one double-broadcast tensor_tensor each. Cascade
                # levels are stored bf16 so the hot final scale-ops stream
                # bf16-in/bf16-out and hit DVE 4x mode (4 elem/cyc/lane).
                nc.vector.tensor_tensor(
                    out=t2.rearrange("p (a b) -> p a b", b=D1),
                    in0=e[:, 20:24, None].broadcast_to((P, D1, D1)),
                    in1=e[:, None, 24:28].broadcast_to((P, D1, D1)),
                    op=Alu.mult,
                )
                nc.vector.tensor_tensor(
                    out=t3.rearrange("p (a b) -> p a b", b=16),
                    in0=e[:, 16:20, None].broadcast_to((P, D1, 16)),
                    in1=t2[:, None, :].broadcast_to((P, D1, 16)),
                    op=Alu.mult,
                )
                nc.vector.tensor_tensor(
                    out=t4r.rearrange("p (a b) -> p a b", b=64),
                    in0=e[:, 12:16, None].broadcast_to((P, D1, 64)),
                    in1=t3[:, None, :].broadcast_to((P, D1, 64)),
                    op=Alu.mult,
                )
                # fold the softmax normalizer C = 1/prod_f Z_f into t4
                nc.vector.reciprocal(out=c1, in_=zp)
                nc.vector.tensor_scalar_mul(out=t4, in0=t4r, scalar1=c1)

                blocks = _BLOCK_PLAN[t]
                lead = [b for b in blocks if b[1] <= 1024]
                for i, (c0, c1_) in enumerate(lead):
                    L = c1_ - c0
                    lb = blkp.tile([P, L], bf16, tag=f"lead{L}")
                    for j in range(L // 256):
                        d2 = c0 // 256 + j
                        q = lb[:, j * 256 : (j + 1) * 256]
                        if d2 >= 2:
                            nc.scalar.mul(out=q, in_=t4, mul=s4[:, d2 : d2 + 1])
                        else:
                            nc.vector.tensor_scalar_mul(
                                out=q, in0=t4, scalar1=s4[:, d2 : d2 + 1]
                            )
                    nc.sync.dma_start(
                        out=out_d[out_off : out_off + P * L].rearrange(
                            "(p l) -> p l", l=L
                        ),
                        in_=lb,
                    )
                    out_off += P * L

                t5 = mp.tile([P, 1024], bf16, tag="t5")
                for d in range(D1):
                    nc.vector.tensor_scalar_mul(
                        out=t5[:, d * 256 : (d + 1) * 256],
                        in0=t4,
                        scalar1=e[:, 8 + d : 9 + d],
                    )
                # remaining blocks: 1024-col units of t5 * sc16-col scale-ops,
                # DMA'd as soon as each block lands. DVE in 4x mode does a
                # 1024-elem unit in ~330ns vs ScalarE's ~1150ns => ~3:1 split.
                for c0, c1_ in blocks[len(lead) :]:
                    L = c1_ - c0
                    blk = blkp.tile([P, L], bf16, tag="blk")
                    for s in range(L // 1024):
                        u = c0 // 1024 + s
                        d0, d1 = u // D1, u % D1
                        scol = sc16[:, d1 * D1 + d0 : d1 * D1 + d0 + 1]
                        q = blk[:, s * 1024 : (s + 1) * 1024]
                        if u in _SCALAR_UNITS[t]:
                            nc.scalar.mul(out=q, in_=t5, mul=scol)
                        else:
                            nc.vector.tensor_scalar_mul(out=q, in0=t5, scalar1=scol)
                    nc.sync.dma_start(
                        out=out_d[out_off : out_off + P * L].rearrange(
                            "(p l) -> p l", l=L
                        ),
                        in_=blk,
                    )
                    out_off += P * L
    # Hoist the input DMA to the top of the entry block, ahead of the
    # const-AP memsets and the entry all-engine barrier. No semaphore-clear
    # exists in this mode (target_bir_lowering=False), so the completion
    # increments on in_sem cannot be wiped.
    entry = nc.main_func.blocks[0]
    dma_idx = next(
        i
        for i, ins in enumerate(entry.instructions)
        if type(ins).__name__ == "InstDMACopy"
    )
    entry.instructions.insert(0, entry.instructions.pop(dma_idx))

    nc.compile()
    return nc


def gather_core(flat):
    """Scatter one core's flat block stream back to its (ROWS, OUT) shard."""
    out = np.empty((ROWS, OUT), dtype=np.float32)
    flat = np.asarray(flat)
    off = 0
    for t, blocks in enumerate(_BLOCK_PLAN):
        for c0, c1 in blocks:
            L = c1 - c0
            out[t * P : (t + 1) * P, c0:c1] = (
                flat[off : off + P * L].reshape(P, L).astype(np.float32)
            )
            off += P * L
    return out


def build_in_maps(x, cutpoints):
    XWC = NTILES * F + F * D1 + F * 3
    XWP = 128
    wpat = np.tile(np.arange(1.0, D1 + 1.0, dtype=np.float32), F)
    cflat = cutpoints.ravel().astype(np.float32)
    # x sharded: core k, partition p gets rows k*512 + {p, 128+p, 256+p, 384+p}
    xs = (
        x.reshape(NCORES, NTILES, P, F)
        .transpose(0, 2, 1, 3)
        .reshape(NCORES, P, NTILES * F)
    )
    in_maps = []
    for k in range(NCORES):
        xw = np.zeros((P, XWP), dtype=np.float32)
        xw[:, 0 : NTILES * F] = xs[k]
        xw[:, NTILES * F : NTILES * F + F * D1] = wpat
        xw[:, NTILES * F + F * D1 : XWC] = cflat
        in_maps.append({"xw": xw})
    return in_maps


def kernel(x, cutpoints):
    from concourse import bass_utils

    if "nc" not in _cache:
        _cache["nc"] = _build_bass()
    nc = _cache["nc"]

    x = np.ascontiguousarray(np.asarray(x), dtype=np.float32)
    cutpoints = np.ascontiguousarray(np.asarray(cutpoints), dtype=np.float32)
    in_maps = build_in_maps(x, cutpoints)
    res = bass_utils.run_bass_kernel_spmd(nc, in_maps, list(range(NCORES))).results
    return np.concatenate([gather_core(res[k]["out"]) for k in range(NCORES)], axis=0)

